# revision 22
# baseline (speedup 1.0000x reference)
"""Trainium2 Bass kernel for nn_CPCircuitLayer.

Math: with all_indices the full cartesian grid (s = n // H, h = n % H),
    out[b, s, h] = sum_r seq_emb[b,s,r] * hid_emb[b,h,r] * cp[r]
                 = (seq_emb[b] @ diag(cp) @ hid_emb[b].T)[s, h]
where seq_emb[b] = X_b @ seq_W.T  (X_b = hidden_states[b], contract H)
      hid_emb[b] = X_b.T @ hid_W.T                        (contract S)

Sharding: 8 cores = (batch b, seq half) pairs. Each core receives X_b in
fp16 (half the HBM bytes of f32) with rows rotated so its own seq half
comes first, plus an fp16 host-transposed copy of that half, and computes
    hid_embT = (hid_W*cp) @ X_b          [R, H]   (contract all 1024 rows)
    seq_embT = seq_W @ X_b[0:512].T      [R, S/2]
    out_half = seq_embT.T @ hid_embT     [S/2, H] written as fp16
The output is upcast to f32 on the host. Per-core HBM traffic:
3 MiB in + 1 MiB out (vs 6 MiB in + 2 MiB out all-f32).

Raw Bass with manual semaphores. DMA instructions cost ~650ns of engine
issue time each, so transfers are batched: x streams as four 512-KiB
two-tile DMAs on the Sync queue (a [8,128,1024] DRAM view makes the
partition-major pair a 3D AP), xt as a single 1-MiB DMA on the Act queue
behind the weights. The PE consumes x pairs as they arrive (hid factor),
slots the 8 seq matmuls behind the xt arrival, runs the final matmuls in
FP32R, and PSUM->SBUF copies (f32 -> fp16 cast for the output) alternate
between Vector and Scalar. Output tiles DMA out on the Sync queue.
A couple of dummy matmuls at kernel start warm the PE HAM clock gate.
"""

import numpy as np

B, S, H, R = 4, 1024, 1024, 32
N_CORES = 8
SH = S // 2     # seq rows per core
KT = S // 128   # k-tiles over the full contraction dims (8)
MT = SH // 128  # row tiles in this core's seq half (4)
NP = KT // 2    # x DMA pair count (4)

_compiled = {}


def _np_fallback(hidden_states, all_indices, seq_W, hid_W, cp_weight):
    seq_emb = np.einsum("bsh,rh->bsr", hidden_states, seq_W)
    hid_emb = np.einsum("bsh,rs->bhr", hidden_states, hid_W)
    s_idx = all_indices[:, 0].astype(np.int64)
    h_idx = all_indices[:, 1].astype(np.int64)
    g_seq = seq_emb[:, s_idx, :]
    g_hid = hid_emb[:, h_idx, :]
    out = np.einsum("bnr,bnr,r->bn", g_seq, g_hid, cp_weight[0])
    return out.reshape(B, S, H).astype(np.float32)


def build_raw_program():
    import contextlib

    import concourse.bass as bass
    import concourse.mybir as mybir

    f32 = mybir.dt.float32
    f32r = mybir.dt.float32r
    f16 = mybir.dt.float16

    nc = bass.Bass("TRN2", target_bir_lowering=False, debug=False,
                   num_devices=N_CORES, enable_partition_id=False)

    # x viewed tile-major so a partition-major pair is a simple 3D AP
    x_d = nc.dram_tensor("x", [KT, 128, H], f16, kind="ExternalInput")
    xt_d = nc.dram_tensor("xt", [128, KT * SH], f16, kind="ExternalInput")
    w_d = nc.dram_tensor("w", [128, 2 * KT * R], f16, kind="ExternalInput")
    out_d = nc.dram_tensor("out", [SH, H], f16, kind="ExternalOutput")

    with contextlib.ExitStack() as _xs:
        E = _xs.enter_context
        w_t = E(nc.sbuf_tensor([128, 2 * KT * R], f16))  # [p, sw | hw]
        x_t = E(nc.sbuf_tensor([128, KT, H], f16))
        xt_t = E(nc.sbuf_tensor([128, KT, SH], f16))     # xT of own half
        hid_sb = E(nc.sbuf_tensor([R, H], f16))
        seq_sb = E(nc.sbuf_tensor([R, SH], f16))
        o_sb = E(nc.sbuf_tensor([128, MT, H], f16))
        hid_ps = E(nc.psum_tensor([R, H], f32))          # 2 banks
        seq_ps = E(nc.psum_tensor([R, SH], f32))         # 1 bank
        o_ps = [E(nc.psum_tensor(f"o_ps{i}", [128, 512], f32))
                for i in range(5)]                       # 5 banks
        # warmup dummies write o_ps[4]; they finish before the final burst
        dum_ps = o_ps[4]
        w_sem = E(nc.semaphore("w_sem"))
        pe_sem = E(nc.semaphore("pe_sem"))
        dve_sem = E(nc.semaphore("dve_sem"))
        act_sem = E(nc.semaphore("act_sem"))
        out_sem = E(nc.semaphore("out_sem"))
        xt_sem = E(nc.semaphore("xt_sem"))
        seqd_sem = E(nc.semaphore("seqd_sem"))
        hidA_sem = E(nc.semaphore("hidA_sem"))
        hidB_sem = E(nc.semaphore("hidB_sem"))
        x_sem = [E(nc.semaphore(f"x_sem{j}")) for j in range(5)]
        block = E(nc.Block(no_gpsimd_drain=True))

        sw = lambda k: w_t.ap()[:, k * R:(k + 1) * R]
        hw = lambda k: w_t.ap()[:, KT * R + k * R:KT * R + (k + 1) * R]

        # x arrives as one 4-tile chunk (x_sem[0]) + four single tiles
        # (x_sem[1..4]) so the tail of the stream feeds the PE immediately.
        # group-end sems: seqd (seq stop), hidA (hid n0 stop), hidB (n1 stop)
        # pe_sem: final mms only (1..8)
        # dve incs: 1 = seq copy, 2 = hid n0 copy, 3.. = even out chunks
        # act incs: 1 = hid n1 copy, 2.. = odd out chunks

        @block.sync
        def _(sync):
            # single FIFO queue in exact consumption order: w, pair0 (hid
            # start), xt (seq), pair1, pair2, then the last two tiles
            # singly so the stream tail feeds the PE immediately
            sync.dma_start(out=w_t.ap(), in_=w_d[:]).then_inc(w_sem, 16)
            sync.dma_start(
                out=x_t.ap()[:, 0:2, :],
                in_=x_d[0:2, :, :].rearrange("t p c -> p t c"),
            ).then_inc(x_sem[0], 16)
            sync.dma_start(
                out=xt_t.ap(), in_=xt_d[:]).then_inc(xt_sem, 16)
            for p in (1, 2):
                sync.dma_start(
                    out=x_t.ap()[:, 2 * p:2 * p + 2, :],
                    in_=x_d[2 * p:2 * p + 2, :, :].rearrange("t p c -> p t c"),
                ).then_inc(x_sem[p], 16)
            for t in (6, 7):
                sync.dma_start(
                    out=x_t.ap()[:, t, :],
                    in_=x_d[t, :, :],
                ).then_inc(x_sem[t - 3], 16)
            # output tiles: dispatch once both chunk copies land in o_sb
            for m in range(MT):
                sync.wait_ge(dve_sem, 3 + m)
                sync.wait_ge(act_sem, 2 + m)
                sync.dma_start(
                    out=out_d[m * 128:(m + 1) * 128, :],
                    in_=o_sb.ap()[:, m, :],
                ).then_inc(out_sem, 16)
            sync.wait_ge(out_sem, 16 * MT)

        @block.tensor
        def _(tensor):
            def dummies(k):
                # HAM warmers: read whatever is in SBUF (values unused),
                # write to a scratch PSUM bank that is never read back
                for _ in range(k):
                    nc.tensor.matmul(dum_ps.ap()[0:R, :], w_t.ap()[:, 0:R],
                                     w_t.ap()[:, 0:512], start=True,
                                     stop=True)

            # gap-free prelude: the HAM 3.4us qualification window resets on
            # any idle gap, so run dummies continuously until pair0 lands
            dummies(9)
            tensor.wait_ge(w_sem, 16)
            dummies(3)

            def hid_tile(t, wait=None):
                if wait is not None:
                    tensor.wait_ge(x_sem[wait], 16)
                for n in range(2):
                    mm = nc.tensor.matmul(
                        hid_ps.ap()[:, n * 512:(n + 1) * 512],
                        hw(t), x_t.ap()[:, t, n * 512:(n + 1) * 512],
                        start=(t == 0), stop=(t == KT - 1),
                    )
                    if t == KT - 1:
                        mm.then_inc(hidA_sem if n == 0 else hidB_sem, 1)

            hid_tile(0, wait=0)
            hid_tile(1)
            dummies(8)                   # keep HAM hot while xt streams
            tensor.wait_ge(xt_sem, 16)
            for kh in range(KT):
                mm = nc.tensor.matmul(
                    seq_ps.ap(), sw(kh), xt_t.ap()[:, kh, :],
                    start=(kh == 0), stop=(kh == KT - 1),
                )
                if kh == KT - 1:
                    mm.then_inc(seqd_sem, 1)
            hid_tile(2, wait=1)
            hid_tile(3)
            hid_tile(4, wait=2)
            hid_tile(5)
            hid_tile(6, wait=3)
            hid_tile(7, wait=4)

            for j in range(2 * MT):      # final: out = seq^T @ hid
                m, n = divmod(j, 2)
                if j == 0:
                    tensor.wait_ge(dve_sem, 2)   # seq + hid n0 copies
                if j == 1:
                    tensor.wait_ge(act_sem, 1)   # hid n1 copy
                if j >= 5:
                    # WAR on recycled PSUM bank (5-deep rotation)
                    prev = j - 5
                    if prev % 2 == 0:
                        tensor.wait_ge(dve_sem, 3 + prev // 2)
                    else:
                        tensor.wait_ge(act_sem, 2 + (prev - 1) // 2)
                nc.tensor.matmul(
                    o_ps[j % 5].ap(),
                    seq_sb.ap()[:, m * 128:(m + 1) * 128],
                    hid_sb.ap()[:, n * 512:(n + 1) * 512],
                    start=True, stop=True,
                ).then_inc(pe_sem, 1)

        @block.vector
        def _(vector):
            vector.wait_ge(seqd_sem, 1)
            nc.vector.tensor_copy(
                seq_sb.ap(), seq_ps.ap()).then_inc(dve_sem, 1)
            vector.wait_ge(hidA_sem, 1)
            nc.vector.tensor_copy(
                hid_sb.ap()[:, 0:512],
                hid_ps.ap()[:, 0:512]).then_inc(dve_sem, 1)
            for j in range(0, 2 * MT, 2):   # even out copies (f32 -> fp16)
                m, n = divmod(j, 2)
                vector.wait_ge(pe_sem, j + 1)
                nc.vector.tensor_copy(
                    o_sb.ap()[:, m, n * 512:(n + 1) * 512],
                    o_ps[j % 5].ap(),
                ).then_inc(dve_sem, 1)

        @block.scalar
        def _(scalar):
            # dummy copy to pull the lazy ACT table load off the critical path
            scalar.wait_ge(w_sem, 16)
            nc.scalar.copy(o_sb.ap()[:, 0, 0:R], w_t.ap()[:, 0:R])
            scalar.wait_ge(hidB_sem, 1)
            nc.scalar.copy(
                hid_sb.ap()[:, 512:1024],
                hid_ps.ap()[:, 512:1024]).then_inc(act_sem, 1)
            for j in range(1, 2 * MT, 2):   # odd out copies (f32 -> fp16)
                m, n = divmod(j, 2)
                scalar.wait_ge(pe_sem, j + 1)
                nc.scalar.copy(
                    o_sb.ap()[:, m, n * 512:(n + 1) * 512],
                    o_ps[j % 5].ap(),
                ).then_inc(act_sem, 1)

    return nc


def _get_program():
    if "nc" not in _compiled:
        _compiled["nc"] = build_raw_program()
    return _compiled["nc"]


def _make_in_maps(hidden_states, seq_W, hid_W, cp_weight):
    # sw[h, r], tiled [128, KT, R] partition-contiguous; hw likewise over s
    swT = np.ascontiguousarray(seq_W.T.astype(np.float16))      # [H, R]
    sw_tiles = swT.reshape(KT, 128, R).transpose(1, 0, 2).reshape(128, KT * R)
    hwT_rows = (hid_W * cp_weight[0][:, None]).T.astype(np.float16)  # [S, R]
    # per-half row rotation: own seq half first (hid contraction over S is
    # order-invariant as long as x rows and hw rows permute together)
    w_rot = []
    for half in range(2):
        hwr = np.concatenate([hwT_rows[half * SH:], hwT_rows[:half * SH]], 0)
        hw_tiles = hwr.reshape(KT, 128, R).transpose(1, 0, 2).reshape(
            128, KT * R)
        w_rot.append(np.ascontiguousarray(
            np.concatenate([sw_tiles, hw_tiles], axis=1)))
    x16 = hidden_states.astype(np.float16)
    in_maps = []
    for c in range(N_CORES):
        b, half = divmod(c, 2)
        xb = x16[b]
        if half:
            xb = np.ascontiguousarray(
                np.concatenate([xb[SH:], xb[:SH]], axis=0))
        # xt[p, kh, c] = xb[c, kh*128 + p] for c in own half
        xt = np.ascontiguousarray(
            xb[:SH].T.reshape(KT, 128, SH).transpose(1, 0, 2)
        ).reshape(128, KT * SH)
        in_maps.append({"x": np.ascontiguousarray(xb).reshape(KT, 128, H),
                        "xt": xt, "w": w_rot[half]})
    return in_maps


def kernel(hidden_states, all_indices, seq_W, hid_W, cp_weight):
    hidden_states = np.asarray(hidden_states, dtype=np.float32)
    seq_W = np.asarray(seq_W, dtype=np.float32)
    hid_W = np.asarray(hid_W, dtype=np.float32)
    cp_weight = np.asarray(cp_weight, dtype=np.float32)
    idx = np.asarray(all_indices)

    # The reference's all_indices is always the full cartesian grid; verify
    # cheaply and fall back to a host path if ever not.
    n = np.arange(S * H, dtype=idx.dtype)
    if idx.shape != (S * H, 2) or not (
        np.array_equal(idx[:, 0], n // H) and np.array_equal(idx[:, 1], n % H)
    ):
        return _np_fallback(hidden_states, idx, seq_W, hid_W, cp_weight)

    from concourse.bass_utils import run_bass_kernel_spmd

    nc = _get_program()
    in_maps = _make_in_maps(hidden_states, seq_W, hid_W, cp_weight)
    res = run_bass_kernel_spmd(nc, in_maps, list(range(N_CORES)))

    out = np.empty((B, S, H), dtype=np.float32)
    for c in range(N_CORES):
        b, half = divmod(c, 2)
        out[b, half * SH:(half + 1) * SH, :] = \
            res.results[c]["out"].astype(np.float32)
    return out


# revision 25
# speedup vs baseline: 1.0725x; 1.0725x over previous
"""Trainium2 Bass kernel for nn_CPCircuitLayer.

Math: with all_indices the full cartesian grid (s = n // H, h = n % H),
    out[b, s, h] = sum_r seq_emb[b,s,r] * hid_emb[b,h,r] * cp[r]
                 = (seq_emb[b] @ diag(cp) @ hid_emb[b].T)[s, h]
where seq_emb[b] = X_b @ seq_W.T  (X_b = hidden_states[b], contract H)
      hid_emb[b] = X_b.T @ hid_W.T                        (contract S)

Sharding: 8 cores = (batch b, seq half) pairs. Each core receives X_b in
fp16 (half the HBM bytes of f32) with rows rotated so its own seq half
comes first, plus an fp16 host-transposed copy of that half, and computes
    hid_embT = (hid_W*cp) @ X_b          [R, H]   (contract all 1024 rows)
    seq_embT = seq_W @ X_b[0:512].T      [R, S/2]
    out_half = seq_embT.T @ hid_embT     [S/2, H] written as fp16
The output is upcast to f32 on the host. Per-core HBM traffic:
3 MiB in + 1 MiB out (vs 6 MiB in + 2 MiB out all-f32).

Raw Bass with manual semaphores. DMA instructions cost ~650ns of engine
issue time each, so transfers are batched: x streams as four 512-KiB
two-tile DMAs on the Sync queue (a [8,128,1024] DRAM view makes the
partition-major pair a 3D AP), xt as a single 1-MiB DMA on the Act queue
behind the weights. The PE consumes x pairs as they arrive (hid factor),
slots the 8 seq matmuls behind the xt arrival, runs the final matmuls in
FP32R, and PSUM->SBUF copies (f32 -> fp16 cast for the output) alternate
between Vector and Scalar. Output tiles DMA out on the Sync queue.
A couple of dummy matmuls at kernel start warm the PE HAM clock gate.
"""

import numpy as np

B, S, H, R = 4, 1024, 1024, 32
N_CORES = 8
SH = S // 2     # seq rows per core
KT = S // 128   # k-tiles over the full contraction dims (8)
MT = SH // 128  # row tiles in this core's seq half (4)
NP = KT // 2    # x DMA pair count (4)

_compiled = {}


def _np_fallback(hidden_states, all_indices, seq_W, hid_W, cp_weight):
    seq_emb = np.einsum("bsh,rh->bsr", hidden_states, seq_W)
    hid_emb = np.einsum("bsh,rs->bhr", hidden_states, hid_W)
    s_idx = all_indices[:, 0].astype(np.int64)
    h_idx = all_indices[:, 1].astype(np.int64)
    g_seq = seq_emb[:, s_idx, :]
    g_hid = hid_emb[:, h_idx, :]
    out = np.einsum("bnr,bnr,r->bn", g_seq, g_hid, cp_weight[0])
    return out.reshape(B, S, H).astype(np.float32)


def build_raw_program():
    import contextlib

    import concourse.bass as bass
    import concourse.mybir as mybir

    f32 = mybir.dt.float32
    f32r = mybir.dt.float32r
    f16 = mybir.dt.float16

    nc = bass.Bass("TRN2", target_bir_lowering=False, debug=False,
                   num_devices=N_CORES, enable_partition_id=False)

    # x viewed tile-major so a partition-major pair is a simple 3D AP
    x_d = nc.dram_tensor("x", [KT, 128, H], f16, kind="ExternalInput")
    xt_d = nc.dram_tensor("xt", [128, KT * SH], f16, kind="ExternalInput")
    w_d = nc.dram_tensor("w", [128, 2 * KT * R], f16, kind="ExternalInput")
    out_d = nc.dram_tensor("out", [SH, H], f16, kind="ExternalOutput")

    with contextlib.ExitStack() as _xs:
        E = _xs.enter_context
        w_t = E(nc.sbuf_tensor([128, 2 * KT * R], f16))  # [p, sw | hw]
        x_t = E(nc.sbuf_tensor([128, KT, H], f16))
        xt_t = E(nc.sbuf_tensor([128, KT, SH], f16))     # xT of own half
        hid_sb = E(nc.sbuf_tensor([R, H], f16))
        seq_sb = E(nc.sbuf_tensor([R, SH], f16))
        o_sb = E(nc.sbuf_tensor([128, MT, H], f16))
        hid_ps = E(nc.psum_tensor([R, H], f32))          # 2 banks
        seq_ps = E(nc.psum_tensor([R, SH], f32))         # 1 bank
        o_ps = [E(nc.psum_tensor(f"o_ps{i}", [128, 512], f32))
                for i in range(5)]                       # 5 banks
        # warmup dummies write o_ps[4]; they finish before the final burst
        dum_ps = o_ps[4]
        w_sem = E(nc.semaphore("w_sem"))
        pe_sem = E(nc.semaphore("pe_sem"))
        dve_sem = E(nc.semaphore("dve_sem"))
        act_sem = E(nc.semaphore("act_sem"))
        out_sem = E(nc.semaphore("out_sem"))
        xt_sem = E(nc.semaphore("xt_sem"))
        seqd_sem = E(nc.semaphore("seqd_sem"))
        hidA_sem = E(nc.semaphore("hidA_sem"))
        hidB_sem = E(nc.semaphore("hidB_sem"))
        x_sem = [E(nc.semaphore(f"x_sem{j}")) for j in range(5)]
        block = E(nc.Block(no_gpsimd_drain=True))

        sw = lambda k: w_t.ap()[:, k * R:(k + 1) * R]
        hw = lambda k: w_t.ap()[:, KT * R + k * R:KT * R + (k + 1) * R]

        # x arrives as one 4-tile chunk (x_sem[0]) + four single tiles
        # (x_sem[1..4]) so the tail of the stream feeds the PE immediately.
        # group-end sems: seqd (seq stop), hidA (hid n0 stop), hidB (n1 stop)
        # pe_sem: final mms only (1..8)
        # dve incs: 1 = seq copy, 2 = hid n0 copy, 3.. = even out chunks
        # act incs: 1 = hid n1 copy, 2.. = odd out chunks

        @block.sync
        def _(sync):
            # In-flight ops on one queue complete together at the stream
            # end (engines round-robin across all queued ops), so split:
            # this queue serializes w + the x tiles (+ later the output
            # tiles); xt streams in parallel on the Act queue.
            sync.dma_start(out=w_t.ap(), in_=w_d[:]).then_inc(w_sem, 16)
            sync.dma_start(
                out=x_t.ap()[:, 0:2, :],
                in_=x_d[0:2, :, :].rearrange("t p c -> p t c"),
            ).then_inc(x_sem[0], 16)
            for p in (1, 2):
                sync.dma_start(
                    out=x_t.ap()[:, 2 * p:2 * p + 2, :],
                    in_=x_d[2 * p:2 * p + 2, :, :].rearrange("t p c -> p t c"),
                ).then_inc(x_sem[p], 16)
            for t in (6, 7):
                sync.dma_start(
                    out=x_t.ap()[:, t, :],
                    in_=x_d[t, :, :],
                ).then_inc(x_sem[t - 3], 16)
            # output tiles: dispatch once both chunk copies land in o_sb
            for m in range(MT):
                sync.wait_ge(dve_sem, 3 + m)
                sync.wait_ge(act_sem, 2 + m)
                sync.dma_start(
                    out=out_d[m * 128:(m + 1) * 128, :],
                    in_=o_sb.ap()[:, m, :],
                ).then_inc(out_sem, 16)
            sync.wait_ge(out_sem, 16 * MT)

        @block.tensor
        def _(tensor):
            def dummies(k):
                # HAM warmers: read whatever is in SBUF (values unused),
                # write to a scratch PSUM bank that is never read back
                for _ in range(k):
                    nc.tensor.matmul(dum_ps.ap()[0:R, :], w_t.ap()[:, 0:R],
                                     w_t.ap()[:, 0:512], start=True,
                                     stop=True)

            # gap-free prelude: the HAM 3.4us qualification window resets on
            # any idle gap, so run dummies continuously until pair0 lands
            dummies(9)
            tensor.wait_ge(w_sem, 16)
            dummies(4)

            def hid_tile(t, wait=None):
                if wait is not None:
                    tensor.wait_ge(x_sem[wait], 16)
                for n in range(2):
                    mm = nc.tensor.matmul(
                        hid_ps.ap()[:, n * 512:(n + 1) * 512],
                        hw(t), x_t.ap()[:, t, n * 512:(n + 1) * 512],
                        start=(t == 0), stop=(t == KT - 1),
                    )
                    if t == KT - 1:
                        mm.then_inc(hidA_sem if n == 0 else hidB_sem, 1)

            hid_tile(0, wait=0)
            hid_tile(1)
            dummies(6)                   # keep HAM hot while xt streams
            tensor.wait_ge(xt_sem, 16)
            for kh in range(KT):
                mm = nc.tensor.matmul(
                    seq_ps.ap(), sw(kh), xt_t.ap()[:, kh, :],
                    start=(kh == 0), stop=(kh == KT - 1),
                )
                if kh == KT - 1:
                    mm.then_inc(seqd_sem, 1)
            hid_tile(2, wait=1)
            hid_tile(3)
            hid_tile(4, wait=2)
            hid_tile(5)
            hid_tile(6, wait=3)
            hid_tile(7, wait=4)

            for j in range(2 * MT):      # final: out = seq^T @ hid
                m, n = divmod(j, 2)
                if j == 0:
                    tensor.wait_ge(dve_sem, 2)   # seq + hid n0 copies
                if j == 1:
                    tensor.wait_ge(act_sem, 1)   # hid n1 copy
                if j >= 5:
                    # WAR on recycled PSUM bank (5-deep rotation)
                    prev = j - 5
                    if prev % 2 == 0:
                        tensor.wait_ge(dve_sem, 3 + prev // 2)
                    else:
                        tensor.wait_ge(act_sem, 2 + (prev - 1) // 2)
                nc.tensor.matmul(
                    o_ps[j % 5].ap(),
                    seq_sb.ap()[:, m * 128:(m + 1) * 128],
                    hid_sb.ap()[:, n * 512:(n + 1) * 512],
                    start=True, stop=True,
                ).then_inc(pe_sem, 1)

        @block.vector
        def _(vector):
            vector.wait_ge(seqd_sem, 1)
            nc.vector.tensor_copy(
                seq_sb.ap(), seq_ps.ap()).then_inc(dve_sem, 1)
            vector.wait_ge(hidA_sem, 1)
            nc.vector.tensor_copy(
                hid_sb.ap()[:, 0:512],
                hid_ps.ap()[:, 0:512]).then_inc(dve_sem, 1)
            for j in range(0, 2 * MT, 2):   # even out copies (f32 -> fp16)
                m, n = divmod(j, 2)
                vector.wait_ge(pe_sem, j + 1)
                nc.vector.tensor_copy(
                    o_sb.ap()[:, m, n * 512:(n + 1) * 512],
                    o_ps[j % 5].ap(),
                ).then_inc(dve_sem, 1)

        @block.scalar
        def _(scalar):
            scalar.dma_start(
                out=xt_t.ap(), in_=xt_d[:]).then_inc(xt_sem, 16)
            # dummy copy to pull the lazy ACT table load off the critical path
            scalar.wait_ge(w_sem, 16)
            nc.scalar.copy(o_sb.ap()[:, 0, 0:R], w_t.ap()[:, 0:R])
            scalar.wait_ge(hidB_sem, 1)
            nc.scalar.copy(
                hid_sb.ap()[:, 512:1024],
                hid_ps.ap()[:, 512:1024]).then_inc(act_sem, 1)
            for j in range(1, 2 * MT, 2):   # odd out copies (f32 -> fp16)
                m, n = divmod(j, 2)
                scalar.wait_ge(pe_sem, j + 1)
                nc.scalar.copy(
                    o_sb.ap()[:, m, n * 512:(n + 1) * 512],
                    o_ps[j % 5].ap(),
                ).then_inc(act_sem, 1)

    return nc


def _get_program():
    if "nc" not in _compiled:
        _compiled["nc"] = build_raw_program()
    return _compiled["nc"]


def _make_in_maps(hidden_states, seq_W, hid_W, cp_weight):
    # sw[h, r], tiled [128, KT, R] partition-contiguous; hw likewise over s
    swT = np.ascontiguousarray(seq_W.T.astype(np.float16))      # [H, R]
    sw_tiles = swT.reshape(KT, 128, R).transpose(1, 0, 2).reshape(128, KT * R)
    hwT_rows = (hid_W * cp_weight[0][:, None]).T.astype(np.float16)  # [S, R]
    # per-half row rotation: own seq half first (hid contraction over S is
    # order-invariant as long as x rows and hw rows permute together)
    w_rot = []
    for half in range(2):
        hwr = np.concatenate([hwT_rows[half * SH:], hwT_rows[:half * SH]], 0)
        hw_tiles = hwr.reshape(KT, 128, R).transpose(1, 0, 2).reshape(
            128, KT * R)
        w_rot.append(np.ascontiguousarray(
            np.concatenate([sw_tiles, hw_tiles], axis=1)))
    x16 = hidden_states.astype(np.float16)
    in_maps = []
    for c in range(N_CORES):
        b, half = divmod(c, 2)
        xb = x16[b]
        if half:
            xb = np.ascontiguousarray(
                np.concatenate([xb[SH:], xb[:SH]], axis=0))
        # xt[p, kh, c] = xb[c, kh*128 + p] for c in own half
        xt = np.ascontiguousarray(
            xb[:SH].T.reshape(KT, 128, SH).transpose(1, 0, 2)
        ).reshape(128, KT * SH)
        in_maps.append({"x": np.ascontiguousarray(xb).reshape(KT, 128, H),
                        "xt": xt, "w": w_rot[half]})
    return in_maps


def kernel(hidden_states, all_indices, seq_W, hid_W, cp_weight):
    hidden_states = np.asarray(hidden_states, dtype=np.float32)
    seq_W = np.asarray(seq_W, dtype=np.float32)
    hid_W = np.asarray(hid_W, dtype=np.float32)
    cp_weight = np.asarray(cp_weight, dtype=np.float32)
    idx = np.asarray(all_indices)

    # The reference's all_indices is always the full cartesian grid; verify
    # cheaply and fall back to a host path if ever not.
    n = np.arange(S * H, dtype=idx.dtype)
    if idx.shape != (S * H, 2) or not (
        np.array_equal(idx[:, 0], n // H) and np.array_equal(idx[:, 1], n % H)
    ):
        return _np_fallback(hidden_states, idx, seq_W, hid_W, cp_weight)

    from concourse.bass_utils import run_bass_kernel_spmd

    nc = _get_program()
    in_maps = _make_in_maps(hidden_states, seq_W, hid_W, cp_weight)
    res = run_bass_kernel_spmd(nc, in_maps, list(range(N_CORES)))

    out = np.empty((B, S, H), dtype=np.float32)
    for c in range(N_CORES):
        b, half = divmod(c, 2)
        out[b, half * SH:(half + 1) * SH, :] = \
            res.results[c]["out"].astype(np.float32)
    return out
